# revision 68
# baseline (speedup 1.0000x reference)
"""Trainium2 Bass kernel for LoopCoderAttention (sparse_attention).

Head-sharded tensor parallelism over 8 NeuronCores:
  core c owns query heads {2c, 2c+1} and KV head c//2.
All on-device tensors live in transposed [feature, token] layout so every
matmul contracts along the partition dim with zero on-device transposes
(except v, which needs one PE transpose per 128-tile).

All matmul operands are bf16 (1 cyc/row on PE, half the HBM traffic of
fp32); PSUM accumulation stays fp32. Attention masks are 0/1 bf16
multiplies applied AFTER exp (all-SBUF bf16 tensor ops hit the DVE 4x
perf mode), the local sliding-window pass runs at 128-query granularity
(2 key tiles per query tile instead of 5x512-wide matmuls), w_o is
prefetched into SBUF during attention, and o_proj is split into hi/lo
token halves so each half starts as soon as its AllToAll lands.

o_proj: a 1MB AllToAll (bf16) reshards attention output from head-sharded
to token-sharded; each core then runs the full 2048-deep contraction for
its 256-token slice (the "all-reduce" happens inside the matmul
accumulation).
"""
import sys
sys.path.insert(0, '/opt/trn_rl_repo')
import numpy as np
import ml_dtypes
import concourse.bass as bass
import concourse.mybir as mybir
import concourse.tile as tile
from concourse import bacc
from concourse.bass_utils import run_bass_kernel_spmd

T = 2048
HID = 2048
HQ = 16
HK = 4
D = 128
WIN = 64
THETA = 10000.0
SCALE = D ** -0.5
NCORES = 8
TCH = 512                 # t-chunk (matmul free dim)
NCH = T // TCH            # 4 chunks
KT = HID // 128           # 16 k-tiles for 2048-deep contractions
ST = T // 128             # 16 s-tiles
TSL = T // NCORES         # 256-token output slice per core

F32 = mybir.dt.float32
BF16 = mybir.dt.bfloat16
AF = mybir.ActivationFunctionType

_CACHE = {}


def _build():
    nc = bacc.Bacc("TRN2", target_bir_lowering=False, debug=False,
                   num_devices=NCORES)
    # HST rows ordered (k, n, p): row (k*4+n)*128+p holds hidden feature
    # k*128+p for tokens [n*512, (n+1)*512)
    HST = nc.dram_tensor("HST", [KT * NCH * 128, TCH], BF16,
                         kind="ExternalInput").ap()
    WQKV = nc.dram_tensor("WQKV", [HID, 512], BF16, kind="ExternalInput").ap()
    KGT = nc.dram_tensor("KGT", [D, T], BF16, kind="ExternalInput").ap()
    VGT = nc.dram_tensor("VGT", [128, ST * D], BF16, kind="ExternalInput").ap()
    WO = nc.dram_tensor("WO", [HID, HID], BF16, kind="ExternalInput").ap()
    WG = nc.dram_tensor("WG", [D, 2], BF16, kind="ExternalInput").ap()
    BG = nc.dram_tensor("BG", [1, 2], F32, kind="ExternalInput").ap()
    CSF = nc.dram_tensor("CSF", [128, T], BF16, kind="ExternalInput").ap()
    SNF = nc.dram_tensor("SNF", [128, T], BF16, kind="ExternalInput").ap()
    ONES = nc.dram_tensor("ONES", [128, 1], BF16, kind="ExternalInput").ap()
    IDN = nc.dram_tensor("IDN", [128, 128], BF16, kind="ExternalInput").ap()
    MASKC = nc.dram_tensor("MASKC", [128, 896], BF16, kind="ExternalInput").ap()
    MASKPD = nc.dram_tensor("MASKPD", [128, 512], BF16, kind="ExternalInput").ap()
    OUT = nc.dram_tensor("OUT", [TSL, HID], F32, kind="ExternalOutput").ap()

    with tile.TileContext(nc) as tc:
        # pools are a strict stack: creation order is the reverse of the
        # release order at each phase boundary
        const = tc.alloc_tile_pool(name="const", bufs=1)
        dram = tc.alloc_tile_pool(name="dram", bufs=1, space="DRAM")
        opool = tc.alloc_tile_pool(name="opool", bufs=1)
        osb = tc.alloc_tile_pool(name="osb", bufs=4)
        work = tc.alloc_tile_pool(name="work", bufs=1)
        ropet = tc.alloc_tile_pool(name="ropet", bufs=2)
        rcpp = tc.alloc_tile_pool(name="rcpp", bufs=7)
        bcp = tc.alloc_tile_pool(name="bcp", bufs=2)
        combp = tc.alloc_tile_pool(name="combp", bufs=3)
        aoutp = tc.alloc_tile_pool(name="aoutp", bufs=3)
        # phase-1-only pools (released together at phase-1 end)
        wqkvp = tc.alloc_tile_pool(name="wqkvp", bufs=1)
        chunkp = tc.alloc_tile_pool(name="chunkp", bufs=2)
        hsp = tc.alloc_tile_pool(name="hsp", bufs=1)
        ps1 = tc.alloc_tile_pool(name="ps1", bufs=7, space="PSUM")

        # warmup all-to-all: absorbs collective rendezvous/ring setup and
        # aligns core skew while phase 1 streams its inputs
        a2aw_i = dram.tile([NCORES, 64], BF16)
        a2aw_o = dram.tile([NCORES, 64], BF16)
        nc.gpsimd.collective_compute(
            "AllToAll", mybir.AluOpType.bypass,
            replica_groups=[list(range(NCORES))],
            ins=[a2aw_i[:].opt()], outs=[a2aw_o[:].opt()])

        # ---- phase-1 inputs first (critical path to first matmul) ----
        wqkv_sb = wqkvp.tile([128, KT, 512], BF16)
        hs0 = hsp.tile([128, KT, TCH], BF16, name="hs0")
        for k in range(KT):
            nc.sync.dma_start(out=wqkv_sb[:, k, :],
                              in_=WQKV[k * 128:(k + 1) * 128, :])
            nc.sync.dma_start(
                out=hs0[:, k, :],
                in_=HST[(k * NCH) * 128:(k * NCH + 1) * 128, :])
        csf_sb = wqkvp.tile([128, T], BF16)
        snf_sb = wqkvp.tile([128, T], BF16)
        idn_sb = wqkvp.tile([128, 128], BF16)
        wg_sb = const.tile([D, 2], BF16)
        bg_sb = const.tile([1, 2], F32)
        kgt_sb = const.tile([D, T], BF16)
        vg_sb = const.tile([128, ST, D], BF16)
        ones_sb = const.tile([128, 1], BF16)
        maskc_sb = const.tile([128, 896], BF16)
        maskpd_sb = const.tile([128, 512], BF16)
        # remaining hs chunks (one big DMA each, packetized across the 16
        # DMA engines) interleaved with rope tables and attention constants
        # so everything lands before its first reader
        hs_lo = {0: hs0}
        hs_big = HST.rearrange("(k n p) x -> n p k x", n=NCH, p=128)
        hs_lo[1] = hsp.tile([128, KT, TCH], BF16, name="hs1")
        nc.sync.dma_start(out=hs_lo[1][:], in_=hs_big[1])
        hs_lo[2] = hsp.tile([128, KT, TCH], BF16, name="hs2")
        nc.sync.dma_start(out=hs_lo[2][:], in_=hs_big[2])
        nc.sync.dma_start(out=csf_sb[:], in_=CSF)
        nc.sync.dma_start(out=snf_sb[:], in_=SNF)
        nc.sync.dma_start(out=idn_sb[:], in_=IDN)
        nc.sync.dma_start(out=wg_sb[:], in_=WG)
        nc.sync.dma_start(out=bg_sb[:], in_=BG)
        hs_lo[3] = hsp.tile([128, KT, TCH], BF16, name="hs3")
        nc.sync.dma_start(out=hs_lo[3][:], in_=hs_big[3])
        nc.sync.dma_start(out=kgt_sb[:], in_=KGT)
        del hs_big
        nc.sync.dma_start(out=vg_sb[:],
                          in_=VGT.rearrange("p (s d) -> p s d", d=D))
        nc.sync.dma_start(out=ones_sb[:], in_=ONES)
        nc.sync.dma_start(out=maskc_sb[:], in_=MASKC)
        nc.sync.dma_start(out=maskpd_sb[:], in_=MASKPD)

        # ---- persistent work tiles (through attention) ----
        qrot = work.tile([128, 2, T], BF16)
        krot = work.tile([128, T], BF16)
        vcur = work.tile([128, ST, D], BF16)   # current v in [s, d] tiles
        # per-head-chunk gate rows, plus 1-gate precomputed in phase 1 so the
        # combine's serial chain skips it
        gates = [work.tile([1, TCH], F32, name=f"gate{r}") for r in range(8)]
        gates1 = [work.tile([1, TCH], F32, name=f"gate1_{r}") for r in range(8)]

        a2a_in = [dram.tile([NCORES, 2 * D, TSL // 2], BF16,
                            name=f"a2ai_{tt}") for tt in range(2)]
        a2a_out = [dram.tile([NCORES, 2 * D, TSL // 2], BF16,
                             name=f"a2ao_{tt}") for tt in range(2)]

        def rope_chunk(dst_full, src, n):
            """dst_full[:, n*TCH:...] = neox-rope of chunk tile src [128, TCH].

            rot = src * [cos;cos] + rot90(src) * [-sin;sin], where rot90 swaps
            the two 64-partition halves (built with two SBUF->SBUF DMAs since
            DVE ops require matching base partitions). Intermediates stay f32;
            only the final add rounds to bf16.
            """
            sl = bass.ds(n * TCH, TCH)
            sr = ropet.tile([128, TCH], F32, tag="ropesr", name=f"sr{n}")
            nc.sync.dma_start(out=sr[0:64, :], in_=src[64:128, :])
            nc.sync.dma_start(out=sr[64:128, :], in_=src[0:64, :])
            ta = ropet.tile([128, TCH], F32, tag="ropetmp", name=f"ra{n}")
            tb = ropet.tile([128, TCH], F32, tag="ropetmp", name=f"rb{n}")
            nc.vector.tensor_mul(ta[:], src[:], csf_sb[:, sl])
            nc.vector.tensor_mul(tb[:], sr[:], snf_sb[:, sl])
            nc.vector.tensor_add(dst_full[:, sl], ta[:], tb[:])

        # ================= phase 1: qkvT = wqkv^T @ hsT =================
        pending_small = []
        for n in range(NCH):
            pss = [ps1.tile([128, TCH], F32, tag="ps1t", name=f"ps1_{n}_{m}")
                   for m in range(4)]
            for k in range(KT):
                hs_t = hs_lo[n][:, k, :]
                for m in range(4):
                    nc.tensor.matmul(pss[m][:],
                                     wqkv_sb[:, k, m * 128:(m + 1) * 128],
                                     hs_t,
                                     start=(k == 0), stop=(k == KT - 1))
            if pending_small:
                pending_small.pop(0)()
            sl = bass.ds(n * TCH, TCH)
            q0c = chunkp.tile([128, TCH], F32, tag="q0c")
            q1c = chunkp.tile([128, TCH], F32, tag="q1c")
            kc = chunkp.tile([128, TCH], F32, tag="kc")
            vc = chunkp.tile([128, TCH], BF16, tag="vc")
            nc.scalar.activation(q0c[:], pss[0][:], AF.Copy)
            nc.scalar.activation(q1c[:], pss[1][:], AF.Copy)
            nc.scalar.activation(kc[:], pss[2][:], AF.Copy)
            nc.vector.tensor_copy(vc[:], pss[3][:])

            rope_chunk(qrot[:, 0, :], q0c, n)
            rope_chunk(qrot[:, 1, :], q1c, n)
            rope_chunk(krot, kc, n)

            def small_ops(n=n, vc=vc, sl=sl):
                # v transposes + gates for chunk n: emitted one chunk later so
                # the PE stream never waits on the DVE rope/copy latency
                for j in range(4):
                    s = 4 * n + j
                    pt = ps1.tile([128, 128], BF16, tag="ps1g",
                                  name=f"pt{s}", bufs=1)
                    nc.tensor.transpose(pt[:], vc[:, j * 128:(j + 1) * 128],
                                        idn_sb[:])
                    nc.vector.tensor_copy(vcur[:, s, :], pt[:])
                for h in range(2):
                    r = 2 * n + h
                    gp = ps1.tile([1, TCH], F32, tag="ps1g",
                                  name=f"gp{r}", bufs=1)
                    nc.tensor.matmul(gp[:], wg_sb[:, h:h + 1],
                                     qrot[:, h, sl], start=True, stop=True)
                    nc.scalar.activation(gates[r][:], gp[:], AF.Sigmoid,
                                         bias=bg_sb[0:1, h:h + 1])
                    nc.vector.tensor_scalar(gates1[r][:], gates[r][:],
                                            -1.0, 1.0,
                                            mybir.AluOpType.mult,
                                            mybir.AluOpType.add)

            pending_small.append(small_ops)

        for f in pending_small:
            f()
        pending_small.clear()

        ps1.release()
        hsp.release()
        chunkp.release()
        wqkvp.release()

        # w_o prefetch into the space freed by phase 1; the 8MB streams in
        # during attention so o_proj never waits on HBM
        wop = tc.alloc_tile_pool(name="wop", bufs=1)
        wo_sb = wop.tile([128, KT, HID], BF16)
        for k in range(KT):
            nc.sync.dma_start(out=wo_sb[:, k, :],
                              in_=WO[k * 128:(k + 1) * 128, :])

        afull_hi = wop.tile([128, KT, TSL // 2], BF16)
        afull_lo = wop.tile([128, KT, TSL // 2], BF16)

        expp = tc.alloc_tile_pool(name="expp", bufs=6)
        explp = tc.alloc_tile_pool(name="explp", bufs=4)
        psqk = tc.alloc_tile_pool(name="psqk", bufs=2, space="PSUM")
        psql = tc.alloc_tile_pool(name="psql", bufs=1, space="PSUM")
        pspv = tc.alloc_tile_pool(name="pspv", bufs=2, space="PSUM")
        pssm = tc.alloc_tile_pool(name="pssm", bufs=2, space="PSUM")
        polo = tc.alloc_tile_pool(name="polo", bufs=1, space="PSUM")

        # o_proj for the LO token half runs interleaved inside chunk 3's
        # attention stream: its matmuls fill the PE's exp-wait slots, so
        # only the HI half remains after the tail all-to-all
        polo_queue = [(e, k) for e in range(NCH) for k in range(KT)]
        polo_state = {}

        def emit_polo(cnt):
            for _ in range(cnt):
                if not polo_queue:
                    return
                e, k = polo_queue.pop(0)
                if k == 0:
                    polo_state["po"] = polo.tile([128, TCH], F32, tag="polo",
                                                 name=f"polo{e}")
                po = polo_state["po"]
                nc.tensor.matmul(po[:], afull_lo[:, k, :],
                                 wo_sb[:, k, e * TCH:(e + 1) * TCH],
                                 start=(k == 0), stop=(k == KT - 1))
                if k == KT - 1:
                    ot = osb.tile([128, TCH], F32, tag="ot", name=f"ot0_{e}")
                    nc.vector.tensor_copy(ot[:], po[:])
                    nc.sync.dma_start(
                        out=OUT[0:128, e * TCH:(e + 1) * TCH], in_=ot[:])

        # ============ phase 2: attention (global + local) ============
        # chunks ascend so the small chunks (0,1) finish first: their
        # all-to-all then has the big chunks' compute as cover for inter-core
        # skew, and the tail all-to-all overlaps o_proj lo.
        # Within a chunk the two heads are INTERLEAVED: every PE instruction
        # has ~6 independent matmuls of cover over its exp dependency, so the
        # tensor engine streams continuously (and stays at full p-state).
        n_ex_alloc = 0
        for n in range(NCH):
            S = 4 * n + 4
            sl = bass.ds(n * TCH, TCH)
            q_ap = [qrot[:, h, sl] for h in range(2)]
            rr = [2 * n, 2 * n + 1]

            # ---- global pass over cached KV (512-wide, causal), one-step
            # software pipeline: qk(s) for both heads, then pv/sum(s-1) ----
            pv_g = [pspv.tile([128, TCH], F32, tag="pv", name=f"pvg{r}")
                    for r in rr]
            sm_g = [pssm.tile([1, TCH], F32, tag="sm", name=f"smg{r}")
                    for r in rr]
            exprev = [None, None]
            for s in range(S + 1):
                excur = [None, None]
                for h in range(2):
                    if s < S:
                        qk = psqk.tile([128, TCH], F32, tag="qk",
                                       name=f"qkg{rr[h]}_{s}")
                        nc.tensor.matmul(qk[:],
                                         kgt_sb[:, s * 128:(s + 1) * 128],
                                         q_ap[h], start=True, stop=True)
                        ex = expp.tile([128, TCH], BF16, tag="ex",
                                       name=f"exg{rr[h]}_{s}")
                        n_ex_alloc += 1
                        j = s - 4 * n
                        # diag tiles: queries x < 128j are fully masked, so
                        # exp can skip them -- the full-width mask multiply
                        # zeroes whatever stale data sits there. Only allowed
                        # once this pool slot has been fully written before
                        # (stale finite exp values; never uninitialized SBUF,
                        # which could hold NaN patterns that survive the *0)
                        xo = 128 * j if (j > 0 and n_ex_alloc > 6) else 0
                        nc.scalar.activation(ex[:, xo:TCH], qk[:, xo:TCH],
                                             AF.Exp, scale=SCALE)
                        if j >= 0:
                            off = (3 - j) * 128
                            exm = expp.tile([128, TCH], BF16, tag="exm",
                                            name=f"exm{rr[h]}_{s}")
                            nc.vector.tensor_mul(exm[:], ex[:],
                                                 maskc_sb[:, off:off + TCH])
                            ex = exm
                        excur[h] = ex
                for h in range(2):
                    if s > 0:
                        first, last = (s == 1), (s == S)
                        nc.tensor.matmul(pv_g[h][:], vg_sb[:, s - 1, :],
                                         exprev[h][:], start=first, stop=last)
                        nc.tensor.matmul(sm_g[h][:], ones_sb[:],
                                         exprev[h][:], start=first, stop=last)
                exprev = excur
                if n == 3:
                    emit_polo(3)
            # free the global-sum PSUM slots early (recip reads PSUM direct)
            rg = [rcpp.tile([1, TCH], F32, tag="rcp", name=f"rg{r}")
                  for r in rr]
            pvgs = [combp.tile([128, TCH], BF16, tag="pvs", name=f"pvgs{r}")
                    for r in rr]
            for h in range(2):
                nc.vector.reciprocal_approx_fast(rg[h][:], sm_g[h][:])
                nc.vector.tensor_copy(pvgs[h][:], pv_g[h][:])

            # ---- local sliding-window pass (128-query tiles, heads
            # interleaved, query tiles paired per PSUM bank) ----
            pv_l = [pspv.tile([128, TCH], F32, tag="pv", name=f"pvl{r}")
                    for r in rr]
            sm_l = [pssm.tile([1, TCH], F32, tag="sm", name=f"sml{r}")
                    for r in rr]
            for jp in range(2):
                qk4 = [psql.tile([128, 512], F32, tag="qkl",
                                 name=f"qkl{r}_{jp}") for r in rr]
                lo = 128 if (n == 0 and jp == 0) else 0
                for h in range(2):
                    for jj in range(2):
                        j = 2 * jp + jj
                        t = 4 * n + j
                        qj = qrot[:, h, t * 128:(t + 1) * 128]
                        off = 256 * jj
                        if t > 0:
                            nc.tensor.matmul(
                                qk4[h][:, off:off + 128],
                                krot[:, (t - 1) * 128:t * 128],
                                qj, start=True, stop=True)
                        nc.tensor.matmul(
                            qk4[h][:, off + 128:off + 256],
                            krot[:, t * 128:(t + 1) * 128],
                            qj, start=True, stop=True)
                ex4m = []
                for h in range(2):
                    ex4 = explp.tile([128, 512], BF16, tag="exl",
                                     name=f"exl{rr[h]}_{jp}")
                    nc.scalar.activation(ex4[:, lo:512], qk4[h][:, lo:512],
                                         AF.Exp, scale=SCALE)
                    exm = explp.tile([128, 512], BF16, tag="exlm",
                                     name=f"exlm{rr[h]}_{jp}")
                    nc.vector.tensor_mul(exm[:, lo:512], ex4[:, lo:512],
                                         maskpd_sb[:, lo:512])
                    ex4m.append(exm)
                for h in range(2):
                    for jj in range(2):
                        j = 2 * jp + jj
                        t = 4 * n + j
                        off = 256 * jj
                        jsl = bass.ds(j * 128, 128)
                        ex = ex4m[h]
                        if t > 0:
                            nc.tensor.matmul(pv_l[h][:, jsl],
                                             vcur[:, t - 1, :],
                                             ex[:, off:off + 128],
                                             start=True, stop=False)
                            nc.tensor.matmul(pv_l[h][:, jsl], vcur[:, t, :],
                                             ex[:, off + 128:off + 256],
                                             start=False, stop=True)
                            nc.tensor.matmul(sm_l[h][:, jsl], ones_sb[:],
                                             ex[:, off:off + 128],
                                             start=True, stop=False)
                            nc.tensor.matmul(sm_l[h][:, jsl], ones_sb[:],
                                             ex[:, off + 128:off + 256],
                                             start=False, stop=True)
                        else:
                            nc.tensor.matmul(pv_l[h][:, jsl], vcur[:, t, :],
                                             ex[:, off + 128:off + 256],
                                             start=True, stop=True)
                            nc.tensor.matmul(sm_l[h][:, jsl], ones_sb[:],
                                             ex[:, off + 128:off + 256],
                                             start=True, stop=True)
                if n == 3:
                    emit_polo(4)
            if n == 3:
                emit_polo(99)

            # ---- combine: out = pv_g*gate/sum_g + pv_l*(1-gate)/sum_l ----
            for h in range(2):
                r = rr[h]
                pvls = combp.tile([128, TCH], BF16, tag="pvs", name=f"pvls{r}")
                nc.vector.tensor_copy(pvls[:], pv_l[h][:])
                rl = rcpp.tile([1, TCH], F32, tag="rcp", name=f"rl{r}")
                ag = rcpp.tile([1, TCH], F32, tag="rcp", name=f"ag{r}")
                al = rcpp.tile([1, TCH], F32, tag="rcp", name=f"al{r}")
                nc.vector.reciprocal_approx_fast(rl[:], sm_l[h][:])
                nc.vector.tensor_mul(ag[:], gates[r][:], rg[h][:])
                nc.vector.tensor_mul(al[:], gates1[r][:], rl[:])
                bg_t = bcp.tile([128, TCH], F32, tag="bcast", name=f"bg_t{r}")
                bl_t = bcp.tile([128, TCH], F32, tag="bcast", name=f"bl_t{r}")
                nc.gpsimd.partition_broadcast(bg_t[:], ag[:])
                nc.gpsimd.partition_broadcast(bl_t[:], al[:])
                t1 = combp.tile([128, TCH], BF16, tag="comb", name=f"t1{r}")
                t2 = combp.tile([128, TCH], BF16, tag="comb", name=f"t2{r}")
                ao = aoutp.tile([128, TCH], BF16, tag="aout", name=f"ao{r}")
                nc.vector.tensor_mul(t1[:], pvgs[h][:], bg_t[:])
                nc.vector.tensor_mul(t2[:], pvls[:], bl_t[:])
                nc.vector.tensor_add(ao[:], t1[:], t2[:])

                # ship finished 128-col blocks to a2a staging
                # token 1024+128c (hi) / 128c (lo) lives in chunk n at column
                # offset 128j; each unit covers 4 destination quarter-blocks
                tt = 1 if n >= 2 else 0
                c0 = (n - 2) * 4 if n >= 2 else n * 4
                for j in range(4):
                    nc.sync.dma_start(
                        out=a2a_in[tt][c0 + j, h * D:(h + 1) * D, :],
                        in_=ao[:, j * 128:(j + 1) * 128])

            if n in (1, 3):
                # all-to-all for this token half (lo overlaps chunks 2,3;
                # hi overlaps the o_proj tail)
                tt = 1 if n >= 2 else 0
                nc.gpsimd.collective_compute(
                    "AllToAll", mybir.AluOpType.bypass,
                    replica_groups=[list(range(NCORES))],
                    ins=[a2a_in[tt][:].opt()],
                    outs=[a2a_out[tt][:].opt()])
                if n == 1:
                    # lo o_proj input gather on the gpsimd queue: one DMA,
                    # placed where its wait on the collective cannot block
                    # the sync DMA FIFO (and resolves before chunk 2's
                    # combine broadcasts need the gpsimd queue)
                    nc.gpsimd.dma_start(
                        out=afull_lo[:],
                        in_=a2a_out[0][:].rearrange("c p n -> (c p) n")
                            .rearrange("(k p) n -> p k n", p=128))

        # hi o_proj input gathers AFTER the loop: their waits on the
        # collective completion semaphore must not sit in the sync DMA FIFO
        # ahead of the chunk 2/3 staging DMAs (that would stall the tail
        # all-to-all). k-tile 2*sc+hh comes from source core sc's head hh
        for k in range(KT):
            nc.sync.dma_start(
                out=afull_hi[:, k, :],
                in_=a2a_out[1][k // 2, (k % 2) * D:(k % 2 + 1) * D, :])

        polo.release()
        pssm.release()
        pspv.release()
        psql.release()
        psqk.release()
        explp.release()
        expp.release()

        # ======== phase 3: o_proj hi half (lo ran inside chunk 3) ========
        # OUT rows 0-127 = low half-slice, rows 128-255 = high half-slice
        pso = tc.alloc_tile_pool(name="pso", bufs=4, space="PSUM")
        pss2 = [pso.tile([128, TCH], F32, tag="po", name=f"po_1_{e}")
                for e in range(NCH)]
        for k in range(KT):
            for e in range(NCH):
                nc.tensor.matmul(pss2[e][:],
                                 afull_hi[:, k, :],
                                 wo_sb[:, k, e * TCH:(e + 1) * TCH],
                                 start=(k == 0), stop=(k == KT - 1))
        for e in range(NCH):
            ot = osb.tile([128, TCH], F32, tag="ot", name=f"ot1_{e}")
            if e % 2 == 0:
                nc.vector.tensor_copy(ot[:], pss2[e][:])
            else:
                nc.scalar.activation(ot[:], pss2[e][:], AF.Copy)
            nc.sync.dma_start(
                out=OUT[128:256, e * TCH:(e + 1) * TCH],
                in_=ot[:])
        pso.release()
        wop.release()
        aoutp.release()
        combp.release()
        bcp.release()
        rcpp.release()
        ropet.release()
        work.release()
        osb.release()
        opool.release()
        dram.release()
        const.release()

    nc.compile()
    return nc


def _host_prep(hidden_states, positions, k_global, v_global, w_qkv, w_o,
               w_gate, b_gate):
    """Layout-only host transforms + constant tables -> per-core in_maps."""
    f32 = np.float32
    bf16 = ml_dtypes.bfloat16
    hs = np.asarray(hidden_states, f32)
    pos = np.asarray(positions)
    kg = np.asarray(k_global, f32)
    vg = np.asarray(v_global, f32)
    wqkv = np.asarray(w_qkv, f32)
    wo = np.ascontiguousarray(np.asarray(w_o, f32).astype(bf16))
    wg = np.asarray(w_gate, f32)
    bg = np.asarray(b_gate, f32)

    # hsT rows ordered (k, n, p) so each phase-1 tile is one contiguous block
    hst = np.ascontiguousarray(
        hs.T.astype(bf16).reshape(KT, 128, NCH, TCH).transpose(0, 2, 1, 3)
        .reshape(KT * NCH * 128, TCH))

    half = D // 2
    inv_freq = (THETA ** (-np.arange(half, dtype=f32) / half)).astype(f32)
    ang = pos.astype(f32)[:, None] * inv_freq[None, :]
    cos_t = np.cos(ang).astype(f32).T       # [64, T]
    sin_t = np.sin(ang).astype(f32).T
    csf = np.ascontiguousarray(np.concatenate([cos_t, cos_t], axis=0).astype(bf16))
    snf = np.ascontiguousarray(np.concatenate([-sin_t, sin_t], axis=0).astype(bf16))

    p = np.arange(128, dtype=np.int64)[:, None]
    # 0/1 multiplicative masks (applied to exp(scores) in bf16)
    # global causal diag-band: tile s=4n+j sliced at offset (3-j)*128
    yc = np.arange(896, dtype=np.int64)[None, :]
    maskc = (yc - p - 384 >= 0).astype(bf16)
    # local paired mask [prev | diag]: prev tile s=t-1 allows k-x >= 128-WIN,
    # diag tile s=t allows 0 <= x-k <= WIN
    x = np.arange(128, dtype=np.int64)[None, :]
    maskd = ((x - p >= 0) & (x - p <= WIN)).astype(bf16)
    maskp = (p - x >= 128 - WIN).astype(bf16)
    maskpd = np.ascontiguousarray(
        np.concatenate([maskp, maskd, maskp, maskd], axis=1))

    ones = np.ones((128, 1), bf16)
    idn = np.eye(128, dtype=bf16)

    in_maps = []
    for c in range(NCORES):
        g = c // 2
        wq = wqkv[:, 2 * c * D:(2 * c + 2) * D]
        wk = wqkv[:, HQ * D + g * D:HQ * D + (g + 1) * D]
        wv = wqkv[:, (HQ + HK) * D + g * D:(HQ + HK) * D + (g + 1) * D]
        vgc = vg[:, g * D:(g + 1) * D]   # [T, D]
        in_maps.append({
            "HST": hst,
            "WQKV": np.ascontiguousarray(
                np.concatenate([wq, wk, wv], axis=1).astype(bf16)),
            "KGT": np.ascontiguousarray(kg[:, g * D:(g + 1) * D].T.astype(bf16)),
            "VGT": np.ascontiguousarray(
                vgc.reshape(ST, 128, D).transpose(1, 0, 2)
                .reshape(128, ST * D).astype(bf16)),
            "WO": wo,
            "WG": np.ascontiguousarray(wg[:, 2 * c:2 * c + 2].astype(bf16)),
            "BG": np.ascontiguousarray(bg[2 * c:2 * c + 2].reshape(1, 2)),
            "CSF": csf,
            "SNF": snf,
            "ONES": ones,
            "IDN": idn,
            "MASKC": maskc,
            "MASKPD": maskpd,
        })
    return in_maps


def kernel(**inputs):
    if "nc" not in _CACHE:
        _CACHE["nc"] = _build()
    nc = _CACHE["nc"]
    in_maps = _host_prep(**inputs)
    res = run_bass_kernel_spmd(nc, in_maps, core_ids=list(range(NCORES)))
    out = np.empty((T, HID), np.float32)
    for c in range(NCORES):
        o = res.results[c]["OUT"]
        out[128 * c:128 * (c + 1)] = o[0:128]
        out[1024 + 128 * c:1024 + 128 * (c + 1)] = o[128:256]
    return out


# revision 73
# speedup vs baseline: 1.0621x; 1.0621x over previous
"""Trainium2 Bass kernel for LoopCoderAttention (sparse_attention).

Head-sharded tensor parallelism over 8 NeuronCores:
  core c owns query heads {2c, 2c+1} and KV head c//2.
All on-device tensors live in transposed [feature, token] layout so every
matmul contracts along the partition dim with zero on-device transposes
(except v, which needs one PE transpose per 128-tile).

All matmul operands are bf16 (1 cyc/row on PE, half the HBM traffic of
fp32); PSUM accumulation stays fp32. Attention masks are 0/1 bf16
multiplies applied AFTER exp (all-SBUF bf16 tensor ops hit the DVE 4x
perf mode), the local sliding-window pass runs at 128-query granularity
(2 key tiles per query tile instead of 5x512-wide matmuls), w_o is
prefetched into SBUF during attention, and o_proj is split into hi/lo
token halves so each half starts as soon as its AllToAll lands.

o_proj: a 1MB AllToAll (bf16) reshards attention output from head-sharded
to token-sharded; each core then runs the full 2048-deep contraction for
its 256-token slice (the "all-reduce" happens inside the matmul
accumulation).
"""
import sys
sys.path.insert(0, '/opt/trn_rl_repo')
import numpy as np
import ml_dtypes
import concourse.bass as bass
import concourse.mybir as mybir
import concourse.tile as tile
from concourse import bacc
from concourse.bass_utils import run_bass_kernel_spmd

T = 2048
HID = 2048
HQ = 16
HK = 4
D = 128
WIN = 64
THETA = 10000.0
SCALE = D ** -0.5
NCORES = 8
TCH = 512                 # t-chunk (matmul free dim)
NCH = T // TCH            # 4 chunks
KT = HID // 128           # 16 k-tiles for 2048-deep contractions
ST = T // 128             # 16 s-tiles
TSL = T // NCORES         # 256-token output slice per core

F32 = mybir.dt.float32
BF16 = mybir.dt.bfloat16
AF = mybir.ActivationFunctionType

_CACHE = {}


def _build():
    nc = bacc.Bacc("TRN2", target_bir_lowering=False, debug=False,
                   num_devices=NCORES)
    # HST rows ordered (k, n, p): row (k*4+n)*128+p holds hidden feature
    # k*128+p for tokens [n*512, (n+1)*512)
    HST = nc.dram_tensor("HST", [KT * NCH * 128, TCH], BF16,
                         kind="ExternalInput").ap()
    WQKV = nc.dram_tensor("WQKV", [HID, 512], BF16, kind="ExternalInput").ap()
    KGT = nc.dram_tensor("KGT", [D, T], BF16, kind="ExternalInput").ap()
    VGT = nc.dram_tensor("VGT", [128, ST * D], BF16, kind="ExternalInput").ap()
    WO = nc.dram_tensor("WO", [HID, HID], BF16, kind="ExternalInput").ap()
    WG = nc.dram_tensor("WG", [D, 2], BF16, kind="ExternalInput").ap()
    BG = nc.dram_tensor("BG", [1, 2], F32, kind="ExternalInput").ap()
    CSF = nc.dram_tensor("CSF", [128, T], BF16, kind="ExternalInput").ap()
    SNF = nc.dram_tensor("SNF", [128, T], BF16, kind="ExternalInput").ap()
    ONES = nc.dram_tensor("ONES", [128, 1], BF16, kind="ExternalInput").ap()
    IDN = nc.dram_tensor("IDN", [128, 128], BF16, kind="ExternalInput").ap()
    MASKC = nc.dram_tensor("MASKC", [128, 896], BF16, kind="ExternalInput").ap()
    MASKPD = nc.dram_tensor("MASKPD", [128, 512], BF16, kind="ExternalInput").ap()
    OUT = nc.dram_tensor("OUT", [TSL, HID], F32, kind="ExternalOutput").ap()

    with tile.TileContext(nc) as tc:
        # pools are a strict stack: creation order is the reverse of the
        # release order at each phase boundary
        const = tc.alloc_tile_pool(name="const", bufs=1)
        dram = tc.alloc_tile_pool(name="dram", bufs=1, space="DRAM")
        opool = tc.alloc_tile_pool(name="opool", bufs=1)
        osb = tc.alloc_tile_pool(name="osb", bufs=4)
        work = tc.alloc_tile_pool(name="work", bufs=1)
        ropet = tc.alloc_tile_pool(name="ropet", bufs=2)
        rcpp = tc.alloc_tile_pool(name="rcpp", bufs=7)
        bcp = tc.alloc_tile_pool(name="bcp", bufs=2)
        combp = tc.alloc_tile_pool(name="combp", bufs=3)
        aoutp = tc.alloc_tile_pool(name="aoutp", bufs=3)
        # phase-1-only pools (released together at phase-1 end)
        wqkvp = tc.alloc_tile_pool(name="wqkvp", bufs=1)
        chunkp = tc.alloc_tile_pool(name="chunkp", bufs=2)
        hsp = tc.alloc_tile_pool(name="hsp", bufs=1)
        ps1 = tc.alloc_tile_pool(name="ps1", bufs=7, space="PSUM")

        # warmup all-to-all: absorbs collective rendezvous/ring setup and
        # aligns core skew while phase 1 streams its inputs
        a2aw_i = dram.tile([NCORES, 64], BF16)
        a2aw_o = dram.tile([NCORES, 64], BF16)
        nc.gpsimd.collective_compute(
            "AllToAll", mybir.AluOpType.bypass,
            replica_groups=[list(range(NCORES))],
            ins=[a2aw_i[:].opt()], outs=[a2aw_o[:].opt()])

        # ---- phase-1 inputs first (critical path to first matmul) ----
        wqkv_sb = wqkvp.tile([128, KT, 512], BF16)
        hs0 = hsp.tile([128, KT, TCH], BF16, name="hs0")
        for k in range(KT):
            nc.sync.dma_start(out=wqkv_sb[:, k, :],
                              in_=WQKV[k * 128:(k + 1) * 128, :])
            nc.sync.dma_start(
                out=hs0[:, k, :],
                in_=HST[(k * NCH) * 128:(k * NCH + 1) * 128, :])
        csf_sb = wqkvp.tile([128, T], BF16)
        snf_sb = wqkvp.tile([128, T], BF16)
        idn_sb = wqkvp.tile([128, 128], BF16)
        wg_sb = const.tile([D, 2], BF16)
        bg_sb = const.tile([1, 2], F32)
        kgt_sb = const.tile([D, T], BF16)
        vg_sb = const.tile([128, ST, D], BF16)
        ones_sb = const.tile([128, 1], BF16)
        maskc_sb = const.tile([128, 896], BF16)
        maskpd_sb = const.tile([128, 512], BF16)
        # remaining hs chunks (one big DMA each, packetized across the 16
        # DMA engines) interleaved with rope tables and attention constants
        # so everything lands before its first reader
        hs_lo = {0: hs0}
        hs_big = HST.rearrange("(k n p) x -> n p k x", n=NCH, p=128)
        hs_lo[1] = hsp.tile([128, KT, TCH], BF16, name="hs1")
        nc.sync.dma_start(out=hs_lo[1][:], in_=hs_big[1])
        hs_lo[2] = hsp.tile([128, KT, TCH], BF16, name="hs2")
        nc.sync.dma_start(out=hs_lo[2][:], in_=hs_big[2])
        nc.sync.dma_start(out=csf_sb[:], in_=CSF)
        nc.sync.dma_start(out=snf_sb[:], in_=SNF)
        nc.sync.dma_start(out=idn_sb[:], in_=IDN)
        nc.sync.dma_start(out=wg_sb[:], in_=WG)
        nc.sync.dma_start(out=bg_sb[:], in_=BG)
        hs_lo[3] = hsp.tile([128, KT, TCH], BF16, name="hs3")
        nc.sync.dma_start(out=hs_lo[3][:], in_=hs_big[3])
        nc.sync.dma_start(out=kgt_sb[:], in_=KGT)
        del hs_big
        nc.sync.dma_start(out=vg_sb[:],
                          in_=VGT.rearrange("p (s d) -> p s d", d=D))
        nc.sync.dma_start(out=ones_sb[:], in_=ONES)
        nc.sync.dma_start(out=maskc_sb[:], in_=MASKC)
        nc.sync.dma_start(out=maskpd_sb[:], in_=MASKPD)

        # ---- persistent work tiles (through attention) ----
        qrot = work.tile([128, 2, T], BF16)
        krot = work.tile([128, T], BF16)
        vcur = work.tile([128, ST, D], BF16)   # current v in [s, d] tiles
        # per-head-chunk gate rows, plus 1-gate precomputed in phase 1 so the
        # combine's serial chain skips it
        gates = [work.tile([1, TCH], F32, name=f"gate{r}") for r in range(8)]
        gates1 = [work.tile([1, TCH], F32, name=f"gate1_{r}") for r in range(8)]

        a2a_in = [dram.tile([NCORES, 2 * D, TSL // 2], BF16,
                            name=f"a2ai_{tt}") for tt in range(2)]
        a2a_out = [dram.tile([NCORES, 2 * D, TSL // 2], BF16,
                             name=f"a2ao_{tt}") for tt in range(2)]

        def rope_chunk(dst_full, src, n):
            """dst_full[:, n*TCH:...] = neox-rope of chunk tile src [128, TCH].

            rot = src * [cos;cos] + rot90(src) * [-sin;sin], where rot90 swaps
            the two 64-partition halves (built with two SBUF->SBUF DMAs since
            DVE ops require matching base partitions). Intermediates stay f32;
            only the final add rounds to bf16.
            """
            sl = bass.ds(n * TCH, TCH)
            sr = ropet.tile([128, TCH], F32, tag="ropesr", name=f"sr{n}")
            nc.sync.dma_start(out=sr[0:64, :], in_=src[64:128, :])
            nc.sync.dma_start(out=sr[64:128, :], in_=src[0:64, :])
            ta = ropet.tile([128, TCH], F32, tag="ropetmp", name=f"ra{n}")
            tb = ropet.tile([128, TCH], F32, tag="ropetmp", name=f"rb{n}")
            nc.vector.tensor_mul(ta[:], src[:], csf_sb[:, sl])
            nc.vector.tensor_mul(tb[:], sr[:], snf_sb[:, sl])
            nc.vector.tensor_add(dst_full[:, sl], ta[:], tb[:])

        # ================= phase 1: qkvT = wqkv^T @ hsT =================
        pending_small = []
        for n in range(NCH):
            pss = [ps1.tile([128, TCH], F32, tag="ps1t", name=f"ps1_{n}_{m}")
                   for m in range(4)]
            for k in range(KT):
                hs_t = hs_lo[n][:, k, :]
                for m in range(4):
                    nc.tensor.matmul(pss[m][:],
                                     wqkv_sb[:, k, m * 128:(m + 1) * 128],
                                     hs_t,
                                     start=(k == 0), stop=(k == KT - 1))
            if pending_small:
                pending_small.pop(0)()
            sl = bass.ds(n * TCH, TCH)
            q0c = chunkp.tile([128, TCH], F32, tag="q0c")
            q1c = chunkp.tile([128, TCH], F32, tag="q1c")
            kc = chunkp.tile([128, TCH], F32, tag="kc")
            vc = chunkp.tile([128, TCH], BF16, tag="vc")
            nc.scalar.activation(q0c[:], pss[0][:], AF.Copy)
            nc.scalar.activation(q1c[:], pss[1][:], AF.Copy)
            nc.scalar.activation(kc[:], pss[2][:], AF.Copy)
            nc.vector.tensor_copy(vc[:], pss[3][:])

            rope_chunk(qrot[:, 0, :], q0c, n)
            rope_chunk(qrot[:, 1, :], q1c, n)
            rope_chunk(krot, kc, n)

            def small_ops(n=n, vc=vc, sl=sl):
                # v transposes + gates for chunk n: emitted one chunk later so
                # the PE stream never waits on the DVE rope/copy latency
                for j in range(4):
                    s = 4 * n + j
                    pt = ps1.tile([128, 128], BF16, tag="ps1g",
                                  name=f"pt{s}", bufs=1)
                    nc.tensor.transpose(pt[:], vc[:, j * 128:(j + 1) * 128],
                                        idn_sb[:])
                    nc.vector.tensor_copy(vcur[:, s, :], pt[:])
                for h in range(2):
                    r = 2 * n + h
                    gp = ps1.tile([1, TCH], F32, tag="ps1g",
                                  name=f"gp{r}", bufs=1)
                    nc.tensor.matmul(gp[:], wg_sb[:, h:h + 1],
                                     qrot[:, h, sl], start=True, stop=True)
                    nc.scalar.activation(gates[r][:], gp[:], AF.Sigmoid,
                                         bias=bg_sb[0:1, h:h + 1])
                    nc.vector.tensor_scalar(gates1[r][:], gates[r][:],
                                            -1.0, 1.0,
                                            mybir.AluOpType.mult,
                                            mybir.AluOpType.add)

            pending_small.append(small_ops)

        for f in pending_small:
            f()
        pending_small.clear()

        ps1.release()
        hsp.release()
        chunkp.release()
        wqkvp.release()

        # w_o prefetch into the space freed by phase 1; the 8MB streams in
        # during attention so o_proj never waits on HBM
        wop = tc.alloc_tile_pool(name="wop", bufs=1)
        wo_sb = wop.tile([128, KT, HID], BF16)
        for k in range(KT):
            nc.sync.dma_start(out=wo_sb[:, k, :],
                              in_=WO[k * 128:(k + 1) * 128, :])

        afull_hi = wop.tile([128, KT, TSL // 2], BF16)
        afull_lo = wop.tile([128, KT, TSL // 2], BF16)

        expp = tc.alloc_tile_pool(name="expp", bufs=6)
        explp = tc.alloc_tile_pool(name="explp", bufs=4)
        psqk = tc.alloc_tile_pool(name="psqk", bufs=2, space="PSUM")
        psql = tc.alloc_tile_pool(name="psql", bufs=2, space="PSUM")
        pspv = tc.alloc_tile_pool(name="pspv", bufs=2, space="PSUM")
        pssm = tc.alloc_tile_pool(name="pssm", bufs=2, space="PSUM")

        # ============ phase 2: attention (global + local) ============
        # chunks ascend so the small chunks (0,1) finish first: their
        # all-to-all then has the big chunks' compute as cover for inter-core
        # skew, and the tail all-to-all overlaps o_proj lo.
        # Within a chunk the two heads are INTERLEAVED: every PE instruction
        # has ~6 independent matmuls of cover over its exp dependency, so the
        # tensor engine streams continuously (and stays at full p-state).
        n_ex_alloc = 0
        for n in range(NCH):
            S = 4 * n + 4
            sl = bass.ds(n * TCH, TCH)
            q_ap = [qrot[:, h, sl] for h in range(2)]
            rr = [2 * n, 2 * n + 1]

            # ---- global pass over cached KV (512-wide, causal), one-step
            # software pipeline: qk(s) for both heads, then pv/sum(s-1) ----
            pv_g = [pspv.tile([128, TCH], F32, tag="pv", name=f"pvg{r}")
                    for r in rr]
            sm_g = [pssm.tile([1, TCH], F32, tag="sm", name=f"smg{r}")
                    for r in rr]
            exprev = [None, None]
            for s in range(S + 1):
                excur = [None, None]
                for h in range(2):
                    if s < S:
                        qk = psqk.tile([128, TCH], F32, tag="qk",
                                       name=f"qkg{rr[h]}_{s}")
                        nc.tensor.matmul(qk[:],
                                         kgt_sb[:, s * 128:(s + 1) * 128],
                                         q_ap[h], start=True, stop=True)
                        ex = expp.tile([128, TCH], BF16, tag="ex",
                                       name=f"exg{rr[h]}_{s}")
                        n_ex_alloc += 1
                        j = s - 4 * n
                        # diag tiles: queries x < 128j are fully masked, so
                        # exp can skip them -- the full-width mask multiply
                        # zeroes whatever stale data sits there. Only allowed
                        # once this pool slot has been fully written before
                        # (stale finite exp values; never uninitialized SBUF,
                        # which could hold NaN patterns that survive the *0)
                        xo = 128 * j if (j > 0 and n_ex_alloc > 6) else 0
                        nc.scalar.activation(ex[:, xo:TCH], qk[:, xo:TCH],
                                             AF.Exp, scale=SCALE)
                        if j >= 0:
                            off = (3 - j) * 128
                            exm = expp.tile([128, TCH], BF16, tag="exm",
                                            name=f"exm{rr[h]}_{s}")
                            nc.vector.tensor_mul(exm[:], ex[:],
                                                 maskc_sb[:, off:off + TCH])
                            ex = exm
                        excur[h] = ex
                for h in range(2):
                    if s > 0:
                        first, last = (s == 1), (s == S)
                        nc.tensor.matmul(pv_g[h][:], vg_sb[:, s - 1, :],
                                         exprev[h][:], start=first, stop=last)
                        nc.tensor.matmul(sm_g[h][:], ones_sb[:],
                                         exprev[h][:], start=first, stop=last)
                exprev = excur
            # free the global-sum PSUM slots early (recip reads PSUM direct)
            rg = [rcpp.tile([1, TCH], F32, tag="rcp", name=f"rg{r}")
                  for r in rr]
            pvgs = [combp.tile([128, TCH], BF16, tag="pvs", name=f"pvgs{r}")
                    for r in rr]
            for h in range(2):
                nc.vector.reciprocal_approx_fast(rg[h][:], sm_g[h][:])
                nc.vector.tensor_copy(pvgs[h][:], pv_g[h][:])

            # ---- local sliding-window pass (128-query tiles, heads
            # interleaved, query tiles paired per PSUM bank) ----
            pv_l = [pspv.tile([128, TCH], F32, tag="pv", name=f"pvl{r}")
                    for r in rr]
            sm_l = [pssm.tile([1, TCH], F32, tag="sm", name=f"sml{r}")
                    for r in rr]
            for jp in range(2):
                qk4 = [psql.tile([128, 512], F32, tag="qkl",
                                 name=f"qkl{r}_{jp}") for r in rr]
                lo = 128 if (n == 0 and jp == 0) else 0
                for h in range(2):
                    for jj in range(2):
                        j = 2 * jp + jj
                        t = 4 * n + j
                        qj = qrot[:, h, t * 128:(t + 1) * 128]
                        off = 256 * jj
                        if t > 0:
                            nc.tensor.matmul(
                                qk4[h][:, off:off + 128],
                                krot[:, (t - 1) * 128:t * 128],
                                qj, start=True, stop=True)
                        nc.tensor.matmul(
                            qk4[h][:, off + 128:off + 256],
                            krot[:, t * 128:(t + 1) * 128],
                            qj, start=True, stop=True)
                ex4m = []
                for h in range(2):
                    ex4 = explp.tile([128, 512], BF16, tag="exl",
                                     name=f"exl{rr[h]}_{jp}")
                    nc.scalar.activation(ex4[:, lo:512], qk4[h][:, lo:512],
                                         AF.Exp, scale=SCALE)
                    exm = explp.tile([128, 512], BF16, tag="exlm",
                                     name=f"exlm{rr[h]}_{jp}")
                    nc.vector.tensor_mul(exm[:, lo:512], ex4[:, lo:512],
                                         maskpd_sb[:, lo:512])
                    ex4m.append(exm)
                for h in range(2):
                    for jj in range(2):
                        j = 2 * jp + jj
                        t = 4 * n + j
                        off = 256 * jj
                        jsl = bass.ds(j * 128, 128)
                        ex = ex4m[h]
                        if t > 0:
                            nc.tensor.matmul(pv_l[h][:, jsl],
                                             vcur[:, t - 1, :],
                                             ex[:, off:off + 128],
                                             start=True, stop=False)
                            nc.tensor.matmul(pv_l[h][:, jsl], vcur[:, t, :],
                                             ex[:, off + 128:off + 256],
                                             start=False, stop=True)
                            nc.tensor.matmul(sm_l[h][:, jsl], ones_sb[:],
                                             ex[:, off:off + 128],
                                             start=True, stop=False)
                            nc.tensor.matmul(sm_l[h][:, jsl], ones_sb[:],
                                             ex[:, off + 128:off + 256],
                                             start=False, stop=True)
                        else:
                            nc.tensor.matmul(pv_l[h][:, jsl], vcur[:, t, :],
                                             ex[:, off + 128:off + 256],
                                             start=True, stop=True)
                            nc.tensor.matmul(sm_l[h][:, jsl], ones_sb[:],
                                             ex[:, off + 128:off + 256],
                                             start=True, stop=True)
            # ---- combine: out = pv_g*gate/sum_g + pv_l*(1-gate)/sum_l ----
            for h in range(2):
                r = rr[h]
                pvls = combp.tile([128, TCH], BF16, tag="pvs", name=f"pvls{r}")
                nc.vector.tensor_copy(pvls[:], pv_l[h][:])
                rl = rcpp.tile([1, TCH], F32, tag="rcp", name=f"rl{r}")
                ag = rcpp.tile([1, TCH], F32, tag="rcp", name=f"ag{r}")
                al = rcpp.tile([1, TCH], F32, tag="rcp", name=f"al{r}")
                nc.vector.reciprocal_approx_fast(rl[:], sm_l[h][:])
                nc.vector.tensor_mul(ag[:], gates[r][:], rg[h][:])
                nc.vector.tensor_mul(al[:], gates1[r][:], rl[:])
                bg_t = bcp.tile([128, TCH], F32, tag="bcast", name=f"bg_t{r}")
                bl_t = bcp.tile([128, TCH], F32, tag="bcast", name=f"bl_t{r}")
                nc.gpsimd.partition_broadcast(bg_t[:], ag[:])
                nc.gpsimd.partition_broadcast(bl_t[:], al[:])
                t1 = combp.tile([128, TCH], BF16, tag="comb", name=f"t1{r}")
                t2 = combp.tile([128, TCH], BF16, tag="comb", name=f"t2{r}")
                ao = aoutp.tile([128, TCH], BF16, tag="aout", name=f"ao{r}")
                nc.vector.tensor_mul(t1[:], pvgs[h][:], bg_t[:])
                nc.vector.tensor_mul(t2[:], pvls[:], bl_t[:])
                nc.vector.tensor_add(ao[:], t1[:], t2[:])

                # ship finished 128-col blocks to a2a staging
                # token 1024+128c (hi) / 128c (lo) lives in chunk n at column
                # offset 128j; each unit covers 4 destination quarter-blocks
                tt = 1 if n >= 2 else 0
                c0 = (n - 2) * 4 if n >= 2 else n * 4
                for j in range(4):
                    nc.sync.dma_start(
                        out=a2a_in[tt][c0 + j, h * D:(h + 1) * D, :],
                        in_=ao[:, j * 128:(j + 1) * 128])

            if n in (1, 3):
                # all-to-all for this token half (lo overlaps chunks 2,3;
                # hi overlaps o_proj lo)
                tt = 1 if n >= 2 else 0
                nc.gpsimd.collective_compute(
                    "AllToAll", mybir.AluOpType.bypass,
                    replica_groups=[list(range(NCORES))],
                    ins=[a2a_in[tt][:].opt()],
                    outs=[a2a_out[tt][:].opt()])

        # o_proj input gathers AFTER the loop: their waits on the collective
        # completion semaphores must not sit in the sync DMA FIFO ahead of
        # the chunk 2/3 staging DMAs (that would stall all-to-all #2).
        # k-tile 2*sc+hh comes from source core sc's head hh
        for tt, afull in ((0, afull_lo), (1, afull_hi)):
            for k in range(KT):
                nc.sync.dma_start(
                    out=afull[:, k, :],
                    in_=a2a_out[tt][k // 2, (k % 2) * D:(k % 2 + 1) * D, :])

        pssm.release()
        pspv.release()
        psql.release()
        psqk.release()
        explp.release()
        expp.release()

        # ============ phase 3: o_proj, lo half then hi half ============
        # OUT rows 0-127 = low half-slice, rows 128-255 = high half-slice
        pso = tc.alloc_tile_pool(name="pso", bufs=8, space="PSUM")
        for tt, afull in ((0, afull_lo), (1, afull_hi)):
            pss2 = [pso.tile([128, TCH], F32, tag="po", name=f"po_{tt}_{e}")
                    for e in range(NCH)]
            for k in range(KT):
                for e in range(NCH):
                    nc.tensor.matmul(pss2[e][:],
                                     afull[:, k, :],
                                     wo_sb[:, k, e * TCH:(e + 1) * TCH],
                                     start=(k == 0), stop=(k == KT - 1))
            for e in range(NCH):
                ot = osb.tile([128, TCH], F32, tag="ot", name=f"ot{tt}_{e}")
                if e % 2 == 0:
                    nc.vector.tensor_copy(ot[:], pss2[e][:])
                else:
                    nc.scalar.activation(ot[:], pss2[e][:], AF.Copy)
                nc.sync.dma_start(
                    out=OUT[tt * 128:(tt + 1) * 128,
                            e * TCH:(e + 1) * TCH],
                    in_=ot[:])
        pso.release()
        wop.release()
        aoutp.release()
        combp.release()
        bcp.release()
        rcpp.release()
        ropet.release()
        work.release()
        osb.release()
        opool.release()
        dram.release()
        const.release()

    nc.compile()
    return nc


def _host_prep(hidden_states, positions, k_global, v_global, w_qkv, w_o,
               w_gate, b_gate):
    """Layout-only host transforms + constant tables -> per-core in_maps."""
    f32 = np.float32
    bf16 = ml_dtypes.bfloat16
    hs = np.asarray(hidden_states, f32)
    pos = np.asarray(positions)
    kg = np.asarray(k_global, f32)
    vg = np.asarray(v_global, f32)
    wqkv = np.asarray(w_qkv, f32)
    wo = np.ascontiguousarray(np.asarray(w_o, f32).astype(bf16))
    wg = np.asarray(w_gate, f32)
    bg = np.asarray(b_gate, f32)

    # hsT rows ordered (k, n, p) so each phase-1 tile is one contiguous block
    hst = np.ascontiguousarray(
        hs.T.astype(bf16).reshape(KT, 128, NCH, TCH).transpose(0, 2, 1, 3)
        .reshape(KT * NCH * 128, TCH))

    half = D // 2
    inv_freq = (THETA ** (-np.arange(half, dtype=f32) / half)).astype(f32)
    ang = pos.astype(f32)[:, None] * inv_freq[None, :]
    cos_t = np.cos(ang).astype(f32).T       # [64, T]
    sin_t = np.sin(ang).astype(f32).T
    csf = np.ascontiguousarray(np.concatenate([cos_t, cos_t], axis=0).astype(bf16))
    snf = np.ascontiguousarray(np.concatenate([-sin_t, sin_t], axis=0).astype(bf16))

    p = np.arange(128, dtype=np.int64)[:, None]
    # 0/1 multiplicative masks (applied to exp(scores) in bf16)
    # global causal diag-band: tile s=4n+j sliced at offset (3-j)*128
    yc = np.arange(896, dtype=np.int64)[None, :]
    maskc = (yc - p - 384 >= 0).astype(bf16)
    # local paired mask [prev | diag]: prev tile s=t-1 allows k-x >= 128-WIN,
    # diag tile s=t allows 0 <= x-k <= WIN
    x = np.arange(128, dtype=np.int64)[None, :]
    maskd = ((x - p >= 0) & (x - p <= WIN)).astype(bf16)
    maskp = (p - x >= 128 - WIN).astype(bf16)
    maskpd = np.ascontiguousarray(
        np.concatenate([maskp, maskd, maskp, maskd], axis=1))

    ones = np.ones((128, 1), bf16)
    idn = np.eye(128, dtype=bf16)

    in_maps = []
    for c in range(NCORES):
        g = c // 2
        wq = wqkv[:, 2 * c * D:(2 * c + 2) * D]
        wk = wqkv[:, HQ * D + g * D:HQ * D + (g + 1) * D]
        wv = wqkv[:, (HQ + HK) * D + g * D:(HQ + HK) * D + (g + 1) * D]
        vgc = vg[:, g * D:(g + 1) * D]   # [T, D]
        in_maps.append({
            "HST": hst,
            "WQKV": np.ascontiguousarray(
                np.concatenate([wq, wk, wv], axis=1).astype(bf16)),
            "KGT": np.ascontiguousarray(kg[:, g * D:(g + 1) * D].T.astype(bf16)),
            "VGT": np.ascontiguousarray(
                vgc.reshape(ST, 128, D).transpose(1, 0, 2)
                .reshape(128, ST * D).astype(bf16)),
            "WO": wo,
            "WG": np.ascontiguousarray(wg[:, 2 * c:2 * c + 2].astype(bf16)),
            "BG": np.ascontiguousarray(bg[2 * c:2 * c + 2].reshape(1, 2)),
            "CSF": csf,
            "SNF": snf,
            "ONES": ones,
            "IDN": idn,
            "MASKC": maskc,
            "MASKPD": maskpd,
        })
    return in_maps


def kernel(**inputs):
    if "nc" not in _CACHE:
        _CACHE["nc"] = _build()
    nc = _CACHE["nc"]
    in_maps = _host_prep(**inputs)
    res = run_bass_kernel_spmd(nc, in_maps, core_ids=list(range(NCORES)))
    out = np.empty((T, HID), np.float32)
    for c in range(NCORES):
        o = res.results[c]["OUT"]
        out[128 * c:128 * (c + 1)] = o[0:128]
        out[1024 + 128 * c:1024 + 128 * (c + 1)] = o[128:256]
    return out
